# revision 22
# baseline (speedup 1.0000x reference)
"""Trainium2 Bass kernel for nn_MultiHeadAttention (B=2, S=2048, D=1024, H=16).

Sharding: 8 cores = 2 (batch) x 4 (head-groups of 4 heads).

Host-side: queries are PERMUTED so unmasked tokens come first; the compacted
key set is then a prefix of xT (no separate xkvT upload). Masked keys are
killed by zeroing their V rows and sums-columns (mask folded into V_ext), so
no exp bias is needed. Output rows are inverse-permuted on host. All device
inputs are pre-packed p-major ([128, ...] partition-first) so every DMA line
is >=2KB contiguous.

Device: QKV^T projections (fp16 matmuls), scores^T flash layout (keys on
partitions). The two heads of a pair run as CONCURRENT row-tiled K=64
matmuls (tile_position row groups 0/64, separate PSUM banks) - scores cost
N cycles for both heads instead of 2N. exp on ScalarE, context accumulated
over key tiles in PSUM with softmax sums via mask-columns in V_ext
(reciprocal reads the sums straight from PSUM). Projection chains and
out-projections are software-pipelined into the attention block stream;
context accumulation is rotated (kt order 2..n-1,0,1) so each block's PSUM
landing zone frees before it is needed. Warmup micro-matmuls cover the
input-DMA wait so the PE HAM clock is at 8/8 when the projection chains
start. (Keepalive filler elsewhere is counterproductive: the power governor
duty-cycles the PE clock to 4/8 under sustained high utilization, so idle
must stay idle.)
"""

import numpy as np

B, S, D = 2, 2048, 1024
NH, DK = 16, 64
SCALE = float(1.0 / np.sqrt(DK))
HPC = 4  # heads per core
P = 128

_NCS = {}
_LAST_PERMS = None


def _build(nkt):
    import concourse.bacc as bacc
    import concourse.mybir as mybir
    import concourse.tile as tile

    F32 = mybir.dt.float32
    F16 = mybir.dt.float16
    MULT = mybir.AluOpType.mult
    ADD = mybir.AluOpType.add
    EXP = mybir.ActivationFunctionType.Exp
    I16 = mybir.dt.int16
    # Schraudolph exp for the DVE half of the split-exp blocks:
    # fp16 bits of ~exp(s*SCALE) = round(s * SCALE*log2e*1024 + 15360)
    SCH_A = SCALE * 1.4426950408889634 * 1024.0
    SCH_B = 15360.0

    NK = nkt * P  # padded key count
    NDT = D // P  # 8 d_model tiles
    NQ = S // 512  # 4 query chunks
    # K-projection chunks of <=512 keys
    KCH = []
    o = 0
    while o < NK:
        KCH.append((o, min(512, NK - o)))
        o += 512

    nc = bacc.Bacc("TRN2", target_bir_lowering=False, debug=False)
    xp_in = nc.dram_tensor("xp", [P, NQ, NDT, 512], F16, kind="ExternalInput")
    wk_in = nc.dram_tensor("wk", [P, NDT, 256], F16, kind="ExternalInput")
    wqv_in = nc.dram_tensor("wqv", [P, NDT, 512], F16, kind="ExternalInput")
    wo_in = nc.dram_tensor("wo", [P, 2, D], F16, kind="ExternalInput")
    sm_in = nc.dram_tensor("smol", [P, 4 + nkt], F32, kind="ExternalInput")
    bvb_in = nc.dram_tensor("bvb", [P, 256], F32, kind="ExternalInput")
    out_dram = nc.dram_tensor("out", [S, D], F16, kind="ExternalOutput")

    with tile.TileContext(nc) as tc:
        from contextlib import ExitStack

        with ExitStack() as ctx:
            pool = ctx.enter_context(tc.tile_pool(name="main", bufs=1))
            pt_pool = ctx.enter_context(tc.tile_pool(name="ptp", bufs=1))
            osb_pool = ctx.enter_context(tc.tile_pool(name="osb", bufs=3))
            sm_pool = ctx.enter_context(tc.tile_pool(name="sm", bufs=2))

            # ---- persistent SBUF tensors ----
            # big [P, NDT, .] tiles so each input loads with ONE dma_start
            # (descriptor-generation on the sync engine is ~0.6us per DMA
            # instruction and serialized); host pre-packs p-major so DMA
            # lines are contiguous.
            xqbig = [
                pool.tile([P, NDT, 512], F16, tag=f"xqbig_{q}", name=f"xqbig_{q}")
                for q in range(4)
            ]
            x16 = [[xqbig[q][:, k, :] for q in range(4)] for k in range(NDT)]
            wkbig = pool.tile([P, NDT, 256], F16, tag="wkbig")
            wqvbig = pool.tile([P, NDT, 512], F16, tag="wqvbig")
            wobig = pool.tile([P, 2, D], F16, tag="wobig")
            wk16 = [wkbig[:, k, :] for k in range(NDT)]
            wq16 = [wqvbig[:, k, 0:256] for k in range(NDT)]
            wv16 = [wqvbig[:, k, 256:512] for k in range(NDT)]
            wo16 = [wobig[:, k, :] for k in range(2)]
            # qT[f]: rows 0:64 = head 2f dims, rows 64:128 = head 2f+1 dims
            qT = [pool.tile([P, S], F16, tag=f"qT_{f}", name=f"qT_{f}") for f in range(2)]
            kT = [pool.tile([P, NK], F16, tag=f"kT_{f}", name=f"kT_{f}") for f in range(2)]
            vext = [pool.tile([P, HPC, 2 * DK], F16, tag=f"vext_{t}", name=f"vext_{t}") for t in range(nkt)]
            ctxT16 = pool.tile([P, 2, S], F16, tag="ctxT16")
            smol = pool.tile([P, 4 + nkt], F32, tag="smol")
            bqk_sb = smol[:, 0:4]
            maskf32 = smol[:, 4 : 4 + nkt]
            ones16 = pool.tile([1, 512], F16, tag="ones16")
            ones3d = pool.tile([P, HPC, DK], F16, tag="ones3d")
            bvb = pool.tile([P, 256], F32, tag="bvb")

            # ---- bulk loads: availability order k-chain0 -> q/v -> rest ----
            nc.gpsimd.memset(ones16[:], 1.0)
            nc.gpsimd.memset(ones3d[:], 1.0)
            nc.sync.dma_start(wkbig[:, 0:2, :], wk_in[:, 0:2, :])
            nc.sync.dma_start(xqbig[0][:, 0:2, :], xp_in[:, 0, 0:2, :])
            nc.sync.dma_start(smol[:], sm_in[:])
            nc.sync.dma_start(wkbig[:, 2:4, :], wk_in[:, 2:4, :])
            nc.sync.dma_start(xqbig[0][:, 2:4, :], xp_in[:, 0, 2:4, :])
            nc.sync.dma_start(wkbig[:, 4:8, :], wk_in[:, 4:8, :])
            nc.sync.dma_start(xqbig[0][:, 4:8, :], xp_in[:, 0, 4:8, :])
            nc.sync.dma_start(bvb[:], bvb_in[:])
            nc.sync.dma_start(wqvbig[:], wqv_in[:])
            for q in range(1, 4):
                nc.sync.dma_start(xqbig[q][:], xp_in[:, q, :, :])
            nc.sync.dma_start(wobig[:], wo_in[:])

            with tc.tile_pool(name="ps_qk", bufs=2, space="PSUM") as ps_qk, tc.tile_pool(
                name="ps_sc", bufs=2, space="PSUM"
            ) as ps_sc, tc.tile_pool(name="ps_ctx", bufs=2, space="PSUM") as ps_ctx:

                # HAM warmup: the PE clock sits at 1.2GHz until ~3.4us of
                # sustained matmul activity. Cover the input-DMA wait with
                # dependency-free matmuls sized so the clock is warm (and the
                # PE free again) right as the first projection data lands.
                warm = ps_qk.tile([P, 512], F32, tag="qkps", name="warm")

                def warmup(n, cols):
                    # dependency-free HAM-feeder matmuls (see module docstring)
                    for _ in range(n):
                        nc.tensor.matmul(
                            warm[:, 0:cols], ones16[0:1, 0:128],
                            ones16[0:1, 0:cols], start=True, stop=True,
                        )

                warmup(12, 256)

                def k_chain(f, c):
                    o, cw = KCH[c]
                    ps = ps_qk.tile([P, 512], F32, tag="qkps", name="kps")
                    for k in range(NDT):
                        nc.tensor.matmul(
                            ps[:, 0:cw],
                            wk16[k][:, f * P : (f + 1) * P],
                            x16[k][o // 512][:, o % 512 : o % 512 + cw],
                            start=(k == 0),
                            stop=(k == NDT - 1),
                        )
                    nc.vector.tensor_scalar_add(
                        kT[f][:, o : o + cw], ps[:, 0:cw], bqk_sb[:, 2 + f : 3 + f]
                    )

                def q_chain(f, t4):
                    ps = ps_qk.tile([P, 512], F32, tag="qkps", name="qps")
                    for k in range(NDT):
                        nc.tensor.matmul(
                            ps[:],
                            wq16[k][:, f * P : (f + 1) * P],
                            x16[k][t4][:],
                            start=(k == 0),
                            stop=(k == NDT - 1),
                        )
                    nc.vector.tensor_scalar_add(
                        qT[f][:, t4 * 512 : (t4 + 1) * 512], ps[:], bqk_sb[:, f : f + 1]
                    )

                def v_chain(t):
                    q, o = (t * P) // 512, (t * P) % 512
                    ps = ps_qk.tile([P, 512], F32, tag="qkps", name="vps")
                    for k in range(NDT):
                        nc.tensor.matmul(
                            ps[:, 0:256],
                            x16[k][q][:, o : o + P],
                            wv16[k][:],
                            start=(k == 0),
                            stop=(k == NDT - 1),
                        )
                    bvbm = sm_pool.tile([P, 256], F32, tag="bvbm", name="bvbm")
                    nc.vector.tensor_scalar_mul(bvbm[:], bvb[:], maskf32[:, t : t + 1])
                    nc.vector.scalar_tensor_tensor(
                        vext[t][:, :, DK : 2 * DK],
                        ps[:, 0:256].rearrange("p (h d) -> p h d", h=HPC),
                        maskf32[:, t : t + 1],
                        bvbm[:].rearrange("p (h d) -> p h d", h=HPC),
                        op0=MULT,
                        op1=ADD,
                    )
                    nc.vector.tensor_scalar(
                        vext[t][:, :, 0:DK], ones3d[:],
                        maskf32[:, t : t + 1], None, op0=MULT,
                    )

                def outproj(qc, evict="v"):
                    outproj_qts(range(qc * 4, qc * 4 + 4), evict)

                def outproj_qts(qts, evict="v", split=False):
                    # qts come in adjacent pairs (qt, qt+1): two query tiles
                    # share one osb + ONE output DMA descriptor, unless
                    # split=True (final drain: per-qt DMAs start sooner).
                    qts = list(qts)
                    step = 1 if split else 2
                    for pi in range(0, len(qts), step):
                        qt0 = qts[pi]
                        osb = osb_pool.tile([P, 2, D], F16, tag="osb", name="osb")
                        for j in range(step):
                            qt = qt0 + j
                            for dmc in range(2):
                                ops = ps_qk.tile([P, 512], F32, tag="qkps", name="ops")
                                for ct in range(2):
                                    nc.tensor.matmul(
                                        ops,
                                        ctxT16[:, ct, qt * P : (qt + 1) * P],
                                        wo16[ct][:, dmc * 512 : (dmc + 1) * 512],
                                        start=(ct == 0),
                                        stop=(ct == 1),
                                    )
                                # "s": evictions on ScalarE; "sv": alternate so
                                # both engines drain the pipeline in parallel
                                if evict == "s" or (evict == "sv" and dmc == 0):
                                    nc.scalar.activation(
                                        osb[:, j, dmc * 512 : (dmc + 1) * 512], ops,
                                        mybir.ActivationFunctionType.Identity,
                                    )
                                else:
                                    nc.vector.tensor_copy(
                                        osb[:, j, dmc * 512 : (dmc + 1) * 512], ops
                                    )
                        nc.sync.dma_start(
                            out_dram[qt0 * P : (qt0 + step) * P, :].rearrange(
                                "(k p) c -> p k c", p=P
                            ),
                            osb[:, 0:step, :],
                        )

                def attn_block(hp, qc, inserts=None, outproj_qc=None, rot=None, tail=False, sch=False):
                    q0 = qc * 512
                    cps = [
                        ps_ctx.tile([P, 512], F32, tag="ctxps", name="ctxps")
                        for _ in range(2)
                    ]
                    pts = [None] * nkt
                    if rot is None:
                        rot = 2 if nkt > 2 else 0  # ctx accumulation starts at kt=rot
                    for kt in range(nkt):
                        scps = ps_sc.tile([P, 1024], F32, tag="scps", name="scps")
                        # the two heads of the pair run CONCURRENTLY: K=64
                        # row-tiled matmuls on array row-groups 0/64 writing
                        # separate PSUM banks.
                        for h2 in range(2):
                            nc.tensor.matmul(
                                scps[:, h2 * 512 : (h2 + 1) * 512],
                                kT[hp][h2 * DK : (h2 + 1) * DK, kt * P : (kt + 1) * P],
                                qT[hp][h2 * DK : (h2 + 1) * DK, q0 : q0 + 512],
                                start=True,
                                stop=True,
                            )
                        pt = pt_pool.tile([P, 1024], F16, tag="pt", bufs=12, name="pt")
                        if sch:
                            # split exp: ScalarE takes head h2=0 exactly, DVE
                            # takes h2=1 via Schraudolph. Each softmax row uses
                            # ONE method for all its keys, so the approx bias
                            # cancels in the normalize.
                            nc.scalar.activation(
                                pt[:, 0:512], scps[:, 0:512], EXP, scale=SCALE
                            )
                            nc.vector.tensor_scalar(
                                pt[:, 512:1024].bitcast(I16), scps[:, 512:1024],
                                SCH_A, SCH_B, op0=MULT, op1=ADD,
                            )
                        else:
                            nc.scalar.activation(pt[:], scps[:], EXP, scale=SCALE)
                        pts[kt] = pt
                        if kt >= rot:
                            for h2 in range(2):
                                nc.tensor.matmul(
                                    cps[h2][:],
                                    vext[kt][:, hp * 2 + h2, :],
                                    pt[:, h2 * 512 : (h2 + 1) * 512],
                                    start=(kt == rot),
                                    stop=(rot == 0 and kt == nkt - 1),
                                )
                        if outproj_qc is not None and kt in (1, 2, 4, 6):
                            # one query-tile per kt slot, front-loaded into the
                            # block-start exp-refill window: evictions (DVE)
                            # hide inside successive exp periods instead of
                            # bursting - a burst delays exp (Scalar FIFO) or
                            # stalls the op matmuls on the ps_qk rotation.
                            opi = {1: 0, 2: 1, 4: 2, 6: 3}[kt]
                            outproj_qts(
                                [outproj_qc * 4 + opi], "v", split=True
                            )
                        if inserts and kt in inserts:
                            for fn in inserts[kt]:
                                fn()
                    for kt in range(rot):  # deferred head of the accumulation
                        for h2 in range(2):
                            nc.tensor.matmul(
                                cps[h2][:],
                                vext[kt][:, hp * 2 + h2, :],
                                pts[kt][:, h2 * 512 : (h2 + 1) * 512],
                                start=False,
                                stop=(kt == rot - 1),
                            )
                    if tail:
                        # tail block: chunked normalize interleaved with the
                        # out-projection so the final drain pipelines across
                        # engines; keepalive micro-matmuls stop the HAM clock
                        # from re-throttling during the short PE-idle drain.
                        recip2 = [
                            sm_pool.tile([DK, 512], F32, tag="recipb", name="recipb")
                            for _ in range(2)
                        ]
                        for half in range(2):
                            c0 = half * 256
                            for h2 in range(2):
                                nc.vector.reciprocal_approx_fast(
                                    recip2[h2][:, c0 : c0 + 256],
                                    cps[h2][0:DK, c0 : c0 + 256],
                                )
                                nc.vector.tensor_tensor(
                                    ctxT16[h2 * DK : (h2 + 1) * DK, hp,
                                           q0 + c0 : q0 + c0 + 256],
                                    cps[h2][DK : 2 * DK, c0 : c0 + 256],
                                    recip2[h2][:, c0 : c0 + 256],
                                    op=MULT,
                                )
                            outproj_qts(
                                [qc * 4 + 2 * half, qc * 4 + 2 * half + 1], "sv",
                                split=(half == 1),
                            )
                    else:
                        for h2 in range(2):
                            recipb = sm_pool.tile([DK, 512], F32, tag="recipb", name="recipb")
                            nc.vector.reciprocal_approx_fast(recipb[:], cps[h2][0:DK, :])
                            nc.vector.tensor_tensor(
                                ctxT16[h2 * DK : (h2 + 1) * DK, hp, q0 : q0 + 512],
                                cps[h2][DK : 2 * DK, :],
                                recipb[:],
                                op=MULT,
                            )

                # ---- ramp: minimum prefix for the first attention block ----
                k_chain(0, 0)
                k_chain(1, 0)
                q_chain(0, 0)
                for t in range(min(4, nkt)):
                    v_chain(t)

                if NQ == 4 and nkt == 9:
                    # chain/outproj inserts sit at kt 1-3 (the block-START
                    # window where the PE otherwise stalls refilling the
                    # scores->exp pipeline), except DMA-gated ones.
                    sched = [
                        # (hp, qc, inserts, outproj_qc, rot)
                        (0, 0, {1: [lambda: k_chain(0, 1)],
                                2: [lambda: v_chain(4)], 3: [lambda: v_chain(5)],
                                4: [lambda: v_chain(6)], 5: [lambda: v_chain(7)],
                                6: [lambda: k_chain(0, 2)], 7: [lambda: v_chain(8)],
                                8: [lambda: q_chain(0, 1)]}, None, None),
                        (0, 1, {1: [lambda: q_chain(0, 2)]}, None, None),
                        (0, 2, {1: [lambda: q_chain(0, 3)], 2: [lambda: k_chain(1, 1)]}, None, None),
                        (0, 3, {1: [lambda: k_chain(1, 2)],
                                2: [lambda: q_chain(1, 0)]}, None, None),
                        (1, 0, {1: [lambda: q_chain(1, 1)]}, None, None),
                        (1, 1, {0: [lambda: q_chain(1, 2)]}, 0, None),
                        (1, 2, {0: [lambda: q_chain(1, 3)]}, 1, None),
                        (1, 3, None, 2, 0),
                    ]
                    for bi, (hp, qc, ins, opq, rot) in enumerate(sched):
                        attn_block(hp, qc, ins, opq, rot, tail=(bi == len(sched) - 1),
                                   sch=False)
                else:  # general fallback: everything sequential
                    for t in range(4, nkt):
                        v_chain(t)
                    for c in range(1, len(KCH)):
                        k_chain(0, c)
                    for t4 in range(1, NQ):
                        q_chain(0, t4)
                    for c in range(1, len(KCH)):
                        k_chain(1, c)
                    for t4 in range(NQ):
                        q_chain(1, t4)
                    for hp in range(2):
                        for qc in range(NQ):
                            attn_block(hp, qc, tail=(hp == 1 and qc == NQ - 1))
                            if hp == 1 and qc < NQ - 1:
                                outproj(qc)

    nc.compile()
    return nc


def _get_nc(nkt=9):
    if nkt not in _NCS:
        _NCS[nkt] = _build(nkt)
    return _NCS[nkt]


def _shard_inputs(x, mask, Wqkv, bqkv, Wout, bout=None):
    global _LAST_PERMS
    x = np.asarray(x, dtype=np.float32)
    mask = np.asarray(mask, dtype=np.int32)
    Wqkv = np.asarray(Wqkv, dtype=np.float32)
    bqkv = np.asarray(bqkv, dtype=np.float32)
    Wout = np.asarray(Wout, dtype=np.float32)

    NDT = D // P
    NQ = S // 512

    # per-batch query permutation: unmasked tokens first
    perms, nks = {}, {}
    for b in range(B):
        idx1 = np.nonzero(mask[b] != 0)[0]
        idx0 = np.nonzero(mask[b] == 0)[0]
        perms[b] = np.concatenate([idx1, idx0])
        nks[b] = len(idx1)
    nkt = max(1, (max(nks.values()) + P - 1) // P)
    NK = nkt * P
    _LAST_PERMS = (perms, nkt)

    def pmajor(w, last):
        # [D, last] -> [P, D//P, last] with row d = k*P + p
        return np.ascontiguousarray(
            w.reshape(NDT, P, last).transpose(1, 0, 2).astype(np.float16)
        )

    xpp, kvm = {}, {}
    for b in range(B):
        xT = x[b].T[:, perms[b]].astype(np.float16)  # [D, S]
        # [P, NQ, NDT, 512]: quarter q, dtile k, 512 cols, contiguous per row
        xpp[b] = np.ascontiguousarray(
            xT.reshape(NDT, P, NQ, 512).transpose(1, 2, 0, 3)
        )
        m = np.zeros(NK, dtype=np.float32)
        m[: nks[b]] = 1.0
        kvm[b] = np.ascontiguousarray(m.reshape(nkt, P).T)  # [P, nkt]

    in_maps = []
    for c in range(8):
        b, hg = divmod(c, 4)
        w0 = hg * 256
        wq = pmajor(Wqkv[:, w0 : w0 + 256], 256)
        wv = pmajor(Wqkv[:, 2 * D + w0 : 2 * D + w0 + 256], 256)
        bqk_pf = (
            np.concatenate([bqkv[w0 : w0 + 256], bqkv[D + w0 : D + w0 + 256]])
            .reshape(4, P)
            .T.astype(np.float32)
        )  # [P, 4]
        smol = np.ascontiguousarray(np.concatenate([bqk_pf, kvm[b]], axis=1))
        bv = bqkv[2 * D + w0 : 2 * D + w0 + 256].astype(np.float32)
        in_maps.append(
            {
                "xp": xpp[b],
                "wk": pmajor(Wqkv[:, D + w0 : D + w0 + 256], 256),
                "wqv": np.ascontiguousarray(np.concatenate([wq, wv], axis=2)),
                "wo": np.ascontiguousarray(
                    Wout[w0 : w0 + 256, :].reshape(2, P, D)
                    .transpose(1, 0, 2).astype(np.float16)
                ),
                "smol": smol,
                "bvb": np.ascontiguousarray(
                    np.broadcast_to(bv[None, :], (P, 256)).astype(np.float32)
                ),
            }
        )
    return in_maps


def kernel(x, mask, Wqkv, bqkv, Wout, bout):
    from concourse.bass_utils import run_bass_kernel_spmd

    in_maps = _shard_inputs(x, mask, Wqkv, bqkv, Wout)
    perms, nkt = _LAST_PERMS
    nc = _get_nc(nkt)
    res = run_bass_kernel_spmd(nc, in_maps, list(range(8))).results
    out = np.zeros((B, S, D), dtype=np.float64)
    for c in range(8):
        b = c // 4
        out[b][perms[b]] += res[c]["out"].astype(np.float64)
    out += np.asarray(bout, dtype=np.float64)[None, None, :]
    return out.astype(np.float32)
